# revision 17
# baseline (speedup 1.0000x reference)
"""Multi-head attention (B=2, S=2048, D=1024, H=16) on 8 NeuronCores.

Sharding: DP(batch) x TP(heads). Core r handles batch r//4 and heads
[4*(r%4), 4*(r%4)+4) as two head-pairs. Per core:
  - q/k/v projections for its batch tokens x its 256 dk columns
  - attention for its 4 heads (head-pair packed matmuls:
      scores: two K=64 matmuls row-packed at partitions 0/64
      PV:     two M=64 matmuls col-packed at psum rows 0/64
      denom:  M=1 ones-matmuls col-packed at psum rows 0/32/64/96)
  - Wo row-slice partial [2048, 1024]
No collectives: the host sums the 4 partials per batch and adds bo.

All matmul operands are bf16 (fp32 PSUM accumulate). exp on ACT as one
[128,1024] instruction spanning 2 psum banks per (pair, sq, sk).
Softmax 1/den via reciprocal_approx_fast batched [97,512].
"""

import sys

sys.path.insert(0, "/opt/trn_rl_repo")

import numpy as np

B, S, D, H, DK = 2, 2048, 1024, 16, 64
NCORES = 8
GC = 4                 # cores per batch group
HPC = 4                # heads per core
DKC = HPC * DK         # 256 dk columns per core
NPAIR = 2              # head pairs per core
KT = D // 128          # 8 contraction tiles for projections
SQB = S // 512         # 4 query blocks
SKT = S // 128         # 16 key tiles

_cache = {}


def _build():
    from contextlib import ExitStack

    from concourse import bacc
    import concourse.mybir as mybir
    import concourse.tile as tile

    f32 = mybir.dt.float32
    bf16 = mybir.dt.bfloat16
    Act = mybir.ActivationFunctionType

    nc = bacc.Bacc(
        "TRN2", target_bir_lowering=False, debug=False,
        enable_asserts=False, num_devices=NCORES,
    )

    xqT = nc.dram_tensor("xqT", [D, S], bf16, kind="ExternalInput").ap()
    xkT = nc.dram_tensor("xkT", [D, S], bf16, kind="ExternalInput").ap()
    xvT = nc.dram_tensor("xvT", [D, S], bf16, kind="ExternalInput").ap()
    wq = nc.dram_tensor("wq", [D, DKC], bf16, kind="ExternalInput").ap()
    wk = nc.dram_tensor("wk", [D, DKC], bf16, kind="ExternalInput").ap()
    wv = nc.dram_tensor("wv", [D, DKC], bf16, kind="ExternalInput").ap()
    wo = nc.dram_tensor("wo", [DKC, D], bf16, kind="ExternalInput").ap()
    bq = nc.dram_tensor("bq", [DKC, 1], f32, kind="ExternalInput").ap()
    bk = nc.dram_tensor("bk", [DKC, 1], f32, kind="ExternalInput").ap()
    bv = nc.dram_tensor("bv", [DKC, 1], f32, kind="ExternalInput").ap()
    out_ext = nc.dram_tensor("out", [S, D], bf16, kind="ExternalOutput").ap()

    with tile.TileContext(nc) as tc, ExitStack() as ctx, \
            nc.allow_low_precision("bf16 matmul operands, fp32 psum accumulate"):
        wpool = ctx.enter_context(tc.tile_pool(name="w", bufs=1))
        xpool = ctx.enter_context(tc.tile_pool(name="x", bufs=24))
        qkpool = ctx.enter_context(tc.tile_pool(name="qk", bufs=1))
        vpool = ctx.enter_context(tc.tile_pool(name="v", bufs=16))
        ptpool = ctx.enter_context(tc.tile_pool(name="pt", bufs=4))
        atpool = ctx.enter_context(tc.tile_pool(name="at", bufs=1))
        smpool = ctx.enter_context(tc.tile_pool(name="sm", bufs=2))
        pvsb = ctx.enter_context(tc.tile_pool(name="pvsb", bufs=4))
        opool = ctx.enter_context(tc.tile_pool(name="o", bufs=3))
        ps_sc = ctx.enter_context(tc.tile_pool(name="pssc", bufs=2, space="PSUM"))
        ps_pv = ctx.enter_context(tc.tile_pool(name="pspv", bufs=2, space="PSUM"))
        ps_dn = ctx.enter_context(tc.tile_pool(name="psdn", bufs=1, space="PSUM"))
        ps_sh = ctx.enter_context(tc.tile_pool(name="pssh", bufs=1, space="PSUM"))

        # ---- input chunks: one big [128, 2048] DMA per (input, k-chunk),
        # issued first on the sync queue so the first matmul isn't stuck
        # behind weight DMAs. Weights go on the (idle) gpsimd queue.
        xk_t, xq_t, xv_t = [], [], []
        for src, lst, eng in ((xkT, xk_t, nc.sync), (xqT, xq_t, nc.scalar),
                              (xvT, xv_t, nc.sync)):
            for k in range(KT):
                t = xpool.tile([128, S], bf16, tag="xt")
                eng.dma_start(t[:], src[k * 128:(k + 1) * 128, :])
                lst.append(t)

        # ---- weights / constants into SBUF ----
        wq_t, wk_t, wv_t = [], [], []
        for name, src, lst in (("wk", wk, wk_t), ("wq", wq, wq_t), ("wv", wv, wv_t)):
            for k in range(KT):
                t = wpool.tile([128, DKC], bf16, tag=f"{name}{k}", name=f"{name}{k}")
                nc.gpsimd.dma_start(t[:], src[k * 128:(k + 1) * 128, :])
                lst.append(t)
        wo_t = []
        for k in range(2):
            t = wpool.tile([128, D], bf16, tag=f"wo{k}", name=f"wo{k}")
            nc.gpsimd.dma_start(t[:], wo[k * 128:(k + 1) * 128, :])
            wo_t.append(t)
        bq_t, bk_t = [], []
        for p in range(NPAIR):
            t = wpool.tile([128, 1], f32, tag=f"bq{p}", name=f"bq{p}")
            nc.gpsimd.dma_start(t[:], bq[p * 128:(p + 1) * 128, :])
            bq_t.append(t)
            t = wpool.tile([128, 1], f32, tag=f"bk{p}", name=f"bk{p}")
            nc.gpsimd.dma_start(t[:], bk[p * 128:(p + 1) * 128, :])
            bk_t.append(t)
        bv_t = []
        for p in range(NPAIR):
            t = wpool.tile([128, 1], f32, tag=f"bv{p}", name=f"bv{p}")
            nc.gpsimd.dma_start(t[:], bv[p * 128:(p + 1) * 128, :])
            bv_t.append(t)

        onesf = wpool.tile([128, 1], f32, tag="onesf")
        nc.gpsimd.memset(onesf[:], 1.0)
        ones_col = wpool.tile([128, 1], bf16, tag="onescol")
        nc.vector.tensor_copy(ones_col[:], onesf[:, 0:1])
        # E97 selectors: rb rows 0:64 <- rec row 64p, rows 64:128 <- row 64p+32
        e97 = []
        for p in range(NPAIR):
            ef = wpool.tile([97, 128], f32, tag=f"e97f{p}", name=f"e97f{p}")
            nc.gpsimd.memset(ef[:], 0.0)
            nc.gpsimd.memset(ef[64 * p:64 * p + 1, 0:64], 1.0)
            nc.gpsimd.memset(ef[64 * p + 32:64 * p + 33, 64:128], 1.0)
            eb = wpool.tile([97, 128], bf16, tag=f"e97_{p}", name=f"e97_{p}")
            nc.vector.tensor_copy(eb[:], ef[:])
            e97.append(eb)

        # ---- k/q projections -> kT_p, qT_p [128, S] bf16 ----
        qT = [qkpool.tile([128, S], bf16, tag=f"qT{p}", name=f"qT{p}")
              for p in range(NPAIR)]
        kTt = [qkpool.tile([128, S], bf16, tag=f"kT{p}", name=f"kT{p}")
               for p in range(NPAIR)]
        # k-outer / blk-inner: 4 consecutive matmuls share one stationary
        # weight tile, and the first matmul only needs the first x chunk
        for xts, w_list, b_list, dsts in (
            (xk_t, wk_t, bk_t, kTt), (xq_t, wq_t, bq_t, qT),
        ):
            for p in range(NPAIR):
                pss = [(ps_pv, "mm"), (ps_dn, "mm"), (ps_sh, "sh"),
                       (ps_pv, "mm")]
                pstiles = [pool.tile([128, 512], f32, tag=tag, name="projps")
                           for pool, tag in pss]
                for k in range(KT):
                    for blk in range(SQB):
                        nc.tensor.matmul(
                            pstiles[blk][:],
                            lhsT=w_list[k][:, p * 128:(p + 1) * 128],
                            rhs=xts[k][:, blk * 512:(blk + 1) * 512],
                            start=(k == 0), stop=(k == KT - 1),
                        )
                for blk in range(SQB):
                    nc.scalar.activation(
                        dsts[p][:, blk * 512:(blk + 1) * 512], pstiles[blk][:],
                        Act.Identity, bias=b_list[p][:, 0:1],
                    )

        # ---- v projection: dk-major vT (N=512 matmuls, weight reuse),
        # then [128,128] DMA-xbar transposes into token-major v tiles ----
        vT = [qkpool.tile([128, S], bf16, tag=f"vT{p}", name=f"vT{p}")
              for p in range(NPAIR)]
        for p in range(NPAIR):
            pss = [(ps_pv, "mm"), (ps_dn, "mm"), (ps_sh, "sh"), (ps_pv, "mm")]
            pstiles = [pool.tile([128, 512], f32, tag=tag, name="vps")
                       for pool, tag in pss]
            for k in range(KT):
                for blk in range(SQB):
                    nc.tensor.matmul(
                        pstiles[blk][:],
                        lhsT=wv_t[k][:, p * 128:(p + 1) * 128],
                        rhs=xv_t[k][:, blk * 512:(blk + 1) * 512],
                        start=(k == 0), stop=(k == KT - 1),
                    )
            for blk in range(SQB):
                nc.scalar.activation(
                    vT[p][:, blk * 512:(blk + 1) * 512], pstiles[blk][:],
                    Act.Identity, bias=bv_t[p][:, 0:1],
                )
        v_t = [vpool.tile([128, DKC], bf16, tag="v", name=f"v{tt}")
               for tt in range(SKT)]
        for tt in range(SKT):
            for p in range(NPAIR):
                eng = nc.sync if (tt + p) % 2 == 0 else nc.scalar
                eng.dma_start_transpose(
                    v_t[tt][:, p * 128:(p + 1) * 128],
                    vT[p][:, tt * 128:(tt + 1) * 128],
                )

        # ---- attention + Wo ----
        attnT = [atpool.tile([128, S], bf16, tag=f"attnT{p}", name=f"attnT{p}")
                 for p in range(NPAIR)]
        wo_pending = []    # (m, n) groups whose attnT deps are satisfied
        norm_pending = []  # deferred normalization closures

        def emit_wo(ms, pspool, pstag):
            for m in ms:
                ot = opool.tile([128, D], bf16, tag="ot", name="ot")
                for n in range(2):
                    ps = pspool.tile([128, 512], f32, tag=pstag, name="wops")
                    for p in range(NPAIR):
                        nc.tensor.matmul(
                            ps[:], lhsT=attnT[p][:, m * 128:(m + 1) * 128],
                            rhs=wo_t[p][:, n * 512:(n + 1) * 512],
                            start=(p == 0), stop=(p == NPAIR - 1),
                        )
                    nc.vector.tensor_copy(ot[:, n * 512:(n + 1) * 512], ps[:])
                nc.sync.dma_start(out_ext[m * 128:(m + 1) * 128, :], ot[:])

        def do_norm(p, sq, pv_sb, rec_b):
            # rb = broadcast of rec rows (64p, 64p+32) over dk rows; then
            # attnT[:, sq block] = pv_sb * rb
            rbp = ps_sh.tile([128, 512], f32, tag="sh", name="rbp")
            nc.tensor.matmul(rbp[:], lhsT=e97[p][:], rhs=rec_b[0:97, :],
                             start=True, stop=True)
            nc.vector.tensor_mul(
                attnT[p][:, sq * 512:(sq + 1) * 512], pv_sb[:], rbp[:])

        def flush_norm():
            while norm_pending:
                do_norm(*norm_pending.pop(0))

        for sq in range(SQB):
            qs = slice(sq * 512, (sq + 1) * 512)
            dn = ps_dn.tile([128, 512], f32, tag="mm", name="dn")
            pv_sbs = []
            for p in range(NPAIR):
                xps = ps_pv.tile([128, 512], f32, tag="mm", name="xps")

                # scores for step sk, emitted 2 steps ahead of their
                # consumers so the ACT exp stream never starves the PE
                # (and vice versa)
                sc_tiles = {}

                def emit_scores(sk):
                    sc = ps_sc.tile([128, 1024], f32, tag="sc", name="sc")
                    for h in range(2):
                        hp = h * 64
                        nc.tensor.matmul(
                            sc[:, h * 512:(h + 1) * 512],
                            lhsT=kTt[p][hp:hp + 64, sk * 128:(sk + 1) * 128],
                            rhs=qT[p][hp:hp + 64, qs],
                            start=True, stop=True,
                        )
                    sc_tiles[sk] = sc

                emit_scores(0)
                emit_scores(1)
                for sk in range(SKT):
                    sc = sc_tiles.pop(sk)
                    pt = ptpool.tile([128, 1024], bf16, tag="pt")
                    nc.scalar.activation(pt[:], sc[:], Act.Exp, scale=0.125)
                    if sk + 2 < SKT:
                        emit_scores(sk + 2)
                    for h in range(2):
                        nc.tensor.matmul(
                            xps[h * 64:(h + 1) * 64, :],
                            lhsT=v_t[sk][:, p * 128 + h * 64:p * 128 + (h + 1) * 64],
                            rhs=pt[:, h * 512:(h + 1) * 512],
                            start=(sk == 0), stop=(sk == SKT - 1),
                        )
                    for h in range(2):
                        r = p * 64 + h * 32
                        nc.tensor.matmul(
                            dn[r:r + 1, :], lhsT=ones_col[:, 0:1],
                            rhs=pt[:, h * 512:(h + 1) * 512],
                            start=(sk == 0), stop=(sk == SKT - 1),
                            tile_position=(0, r),
                        )
                    # deferred work from the previous sq, placed where its
                    # inputs are long since ready so the PE never stalls
                    if sk == 4:
                        flush_norm()
                    if sk >= 8 and wo_pending:
                        emit_wo([wo_pending.pop(0)], ps_sh, "sh")
                # copy PV out of psum promptly so the next sq can reuse it
                pv_sb = pvsb.tile([128, 512], f32, tag="pvsb", name="pvsb")
                nc.vector.tensor_copy(pv_sb[:], xps[:])
                pv_sbs.append(pv_sb)

            # denominators -> batched fast reciprocal (both pairs at once)
            den_sb = smpool.tile([97, 512], f32, tag="densb", name="densb")
            nc.vector.tensor_copy(den_sb[:], dn[0:97, :])
            rec_f = smpool.tile([97, 512], f32, tag="recf", name="recf")
            nc.vector.reciprocal_approx_fast(rec_f[:], den_sb[:])
            rec_b = smpool.tile([97, 512], bf16, tag="recb", name="recb")
            nc.vector.tensor_copy(rec_b[:], rec_f[:])
            for p in range(NPAIR):
                norm_pending.append((p, sq, pv_sbs[p], rec_b))

            wo_pending.extend(range(sq * 4, (sq + 1) * 4))

        # drain: last sq's normalization + remaining Wo via the (now idle)
        # scores pool for 2-deep pipelining
        flush_norm()
        emit_wo(wo_pending, ps_sc, "sc")

    nc.compile()
    return nc


def _get_nc():
    if "nc" not in _cache:
        _cache["nc"] = _build()
    return _cache["nc"]


def kernel(query, key, value, Wq, bq, Wk, bk, Wv, bv, Wo, bo, trace=False):
    import ml_dtypes
    from concourse.bass_utils import run_bass_kernel_spmd

    nc = _get_nc()
    bf = ml_dtypes.bfloat16

    q = np.asarray(query, np.float32)
    k = np.asarray(key, np.float32)
    v = np.asarray(value, np.float32)
    xT = {}
    for nm, x in (("q", q), ("k", k), ("v", v)):
        for b in range(B):
            xT[(nm, b)] = np.ascontiguousarray(x[b].T).astype(bf)
    Wq = np.asarray(Wq, np.float32).astype(bf)
    Wk = np.asarray(Wk, np.float32).astype(bf)
    Wv = np.asarray(Wv, np.float32).astype(bf)
    Wo = np.asarray(Wo, np.float32).astype(bf)
    bqf = np.asarray(bq, np.float32)
    bkf = np.asarray(bk, np.float32)
    bvf = np.asarray(bv, np.float32)

    in_maps = []
    for r in range(NCORES):
        b, g = divmod(r, GC)
        sl = slice(g * DKC, (g + 1) * DKC)
        in_maps.append({
            "xqT": xT[("q", b)], "xkT": xT[("k", b)], "xvT": xT[("v", b)],
            "wq": np.ascontiguousarray(Wq[:, sl]),
            "wk": np.ascontiguousarray(Wk[:, sl]),
            "wv": np.ascontiguousarray(Wv[:, sl]),
            "wo": np.ascontiguousarray(Wo[sl, :]),
            "bq": np.ascontiguousarray(bqf[sl, None]),
            "bk": np.ascontiguousarray(bkf[sl, None]),
            "bv": np.ascontiguousarray(bvf[sl, None]),
        })

    res = run_bass_kernel_spmd(nc, in_maps, list(range(NCORES)), trace=trace)
    _cache["last_results"] = res

    bo = np.asarray(bo, np.float32)
    out = np.empty((B, S, D), np.float32)
    for b in range(B):
        acc = np.zeros((S, D), np.float32)
        for g in range(GC):
            acc += np.asarray(res.results[b * GC + g]["out"], np.float32)
        out[b] = acc + bo[None, :]
    return out


# revision 22
# speedup vs baseline: 1.0566x; 1.0566x over previous
"""Multi-head attention (B=2, S=2048, D=1024, H=16) on 8 NeuronCores.

Sharding: DP(batch) x TP(heads). Core r handles batch r//4 and heads
[4*(r%4), 4*(r%4)+4) as two head-pairs. Per core:
  - q/k/v projections for its batch tokens x its 256 dk columns
  - attention for its 4 heads (head-pair packed matmuls:
      scores: two K=64 matmuls row-packed at partitions 0/64
      PV:     two M=64 matmuls col-packed at psum rows 0/64
      denom:  M=1 ones-matmuls col-packed at psum rows 0/32/64/96)
  - Wo row-slice partial [2048, 1024]
No collectives: the host sums the 4 partials per batch and adds bo.

All matmul operands are bf16 (fp32 PSUM accumulate). exp on ACT as one
[128,1024] instruction spanning 2 psum banks per (pair, sq, sk).
Softmax 1/den via reciprocal_approx_fast batched [97,512].
"""

import sys

sys.path.insert(0, "/opt/trn_rl_repo")

import numpy as np

B, S, D, H, DK = 2, 2048, 1024, 16, 64
NCORES = 8
GC = 4                 # cores per batch group
HPC = 4                # heads per core
DKC = HPC * DK         # 256 dk columns per core
NPAIR = 2              # head pairs per core
KT = D // 128          # 8 contraction tiles for projections
SQB = S // 512         # 4 query blocks
SKT = S // 128         # 16 key tiles

_cache = {}


def _build():
    from contextlib import ExitStack

    from concourse import bacc
    import concourse.mybir as mybir
    import concourse.tile as tile

    f32 = mybir.dt.float32
    bf16 = mybir.dt.bfloat16
    Act = mybir.ActivationFunctionType

    nc = bacc.Bacc(
        "TRN2", target_bir_lowering=False, debug=False,
        enable_asserts=False, num_devices=NCORES,
    )

    xqT = nc.dram_tensor("xqT", [D, S], bf16, kind="ExternalInput").ap()
    xkT = nc.dram_tensor("xkT", [D, S], bf16, kind="ExternalInput").ap()
    xvT = nc.dram_tensor("xvT", [D, S], bf16, kind="ExternalInput").ap()
    wq = nc.dram_tensor("wq", [D, DKC], bf16, kind="ExternalInput").ap()
    wk = nc.dram_tensor("wk", [D, DKC], bf16, kind="ExternalInput").ap()
    wv = nc.dram_tensor("wv", [D, DKC], bf16, kind="ExternalInput").ap()
    wo = nc.dram_tensor("wo", [DKC, D], bf16, kind="ExternalInput").ap()
    bq = nc.dram_tensor("bq", [DKC, 1], f32, kind="ExternalInput").ap()
    bk = nc.dram_tensor("bk", [DKC, 1], f32, kind="ExternalInput").ap()
    bv = nc.dram_tensor("bv", [1, DKC], bf16, kind="ExternalInput").ap()
    out_ext = nc.dram_tensor("out", [S, D], bf16, kind="ExternalOutput").ap()

    with tile.TileContext(nc) as tc, ExitStack() as ctx, \
            nc.allow_low_precision("bf16 matmul operands, fp32 psum accumulate"):
        wpool = ctx.enter_context(tc.tile_pool(name="w", bufs=1))
        xpool = ctx.enter_context(tc.tile_pool(name="x", bufs=24))
        qkpool = ctx.enter_context(tc.tile_pool(name="qk", bufs=1))
        vpool = ctx.enter_context(tc.tile_pool(name="v", bufs=16))
        ptpool = ctx.enter_context(tc.tile_pool(name="pt", bufs=4))
        atpool = ctx.enter_context(tc.tile_pool(name="at", bufs=1))
        smpool = ctx.enter_context(tc.tile_pool(name="sm", bufs=2))
        pvsb = ctx.enter_context(tc.tile_pool(name="pvsb", bufs=4))
        opool = ctx.enter_context(tc.tile_pool(name="o", bufs=3))
        ps_sc = ctx.enter_context(tc.tile_pool(name="pssc", bufs=2, space="PSUM"))
        ps_pv = ctx.enter_context(tc.tile_pool(name="pspv", bufs=2, space="PSUM"))
        ps_dn = ctx.enter_context(tc.tile_pool(name="psdn", bufs=1, space="PSUM"))
        ps_sh = ctx.enter_context(tc.tile_pool(name="pssh", bufs=1, space="PSUM"))

        # ---- input chunks: one big [128, 2048] DMA per (input, k-chunk),
        # issued first on the sync queue so the first matmul isn't stuck
        # behind weight DMAs. Weights go on the (idle) gpsimd queue.
        xk_t, xq_t, xv_t = [], [], []
        for src, lst, eng in ((xkT, xk_t, nc.sync), (xqT, xq_t, nc.scalar),
                              (xvT, xv_t, nc.sync)):
            for k in range(KT):
                t = xpool.tile([128, S], bf16, tag="xt")
                eng.dma_start(t[:], src[k * 128:(k + 1) * 128, :])
                lst.append(t)

        # ---- weights / constants into SBUF ----
        wq_t, wk_t, wv_t = [], [], []
        for name, src, lst in (("wk", wk, wk_t), ("wq", wq, wq_t), ("wv", wv, wv_t)):
            for k in range(KT):
                t = wpool.tile([128, DKC], bf16, tag=f"{name}{k}", name=f"{name}{k}")
                nc.gpsimd.dma_start(t[:], src[k * 128:(k + 1) * 128, :])
                lst.append(t)
        wo_t = []
        for k in range(2):
            t = wpool.tile([128, D], bf16, tag=f"wo{k}", name=f"wo{k}")
            nc.gpsimd.dma_start(t[:], wo[k * 128:(k + 1) * 128, :])
            wo_t.append(t)
        bq_t, bk_t = [], []
        for p in range(NPAIR):
            t = wpool.tile([128, 1], f32, tag=f"bq{p}", name=f"bq{p}")
            nc.gpsimd.dma_start(t[:], bq[p * 128:(p + 1) * 128, :])
            bq_t.append(t)
            t = wpool.tile([128, 1], f32, tag=f"bk{p}", name=f"bk{p}")
            nc.gpsimd.dma_start(t[:], bk[p * 128:(p + 1) * 128, :])
            bk_t.append(t)
        bv_t = wpool.tile([1, DKC], bf16, tag="bv")
        nc.gpsimd.dma_start(bv_t[:], bv[:])

        onesf = wpool.tile([128, 128], f32, tag="onesf")
        nc.gpsimd.memset(onesf[:], 1.0)
        ones_col = wpool.tile([128, 1], bf16, tag="onescol")
        nc.vector.tensor_copy(ones_col[:], onesf[:, 0:1])
        ones_row = wpool.tile([1, 128], bf16, tag="onesrow")
        nc.vector.tensor_copy(ones_row[:], onesf[0:1, :])
        # E97 selectors: rb rows 0:64 <- rec row 64p, rows 64:128 <- row 64p+32
        e97 = []
        for p in range(NPAIR):
            ef = wpool.tile([97, 128], f32, tag=f"e97f{p}", name=f"e97f{p}")
            nc.gpsimd.memset(ef[:], 0.0)
            nc.gpsimd.memset(ef[64 * p:64 * p + 1, 0:64], 1.0)
            nc.gpsimd.memset(ef[64 * p + 32:64 * p + 33, 64:128], 1.0)
            eb = wpool.tile([97, 128], bf16, tag=f"e97_{p}", name=f"e97_{p}")
            nc.vector.tensor_copy(eb[:], ef[:])
            e97.append(eb)

        # ---- k/q projections -> kT_p, qT_p [128, S] bf16 ----
        qT = [qkpool.tile([128, S], bf16, tag=f"qT{p}", name=f"qT{p}")
              for p in range(NPAIR)]
        kTt = [qkpool.tile([128, S], bf16, tag=f"kT{p}", name=f"kT{p}")
               for p in range(NPAIR)]
        # k-outer / blk-inner: 4 consecutive matmuls share one stationary
        # weight tile, and the first matmul only needs the first x chunk
        for xts, w_list, b_list, dsts in (
            (xk_t, wk_t, bk_t, kTt), (xq_t, wq_t, bq_t, qT),
        ):
            for p in range(NPAIR):
                pss = [(ps_pv, "mm"), (ps_dn, "mm"), (ps_sh, "sh"),
                       (ps_pv, "mm")]
                pstiles = [pool.tile([128, 512], f32, tag=tag, name="projps")
                           for pool, tag in pss]
                for k in range(KT):
                    for blk in range(SQB):
                        nc.tensor.matmul(
                            pstiles[blk][:],
                            lhsT=w_list[k][:, p * 128:(p + 1) * 128],
                            rhs=xts[k][:, blk * 512:(blk + 1) * 512],
                            start=(k == 0), stop=(k == KT - 1),
                        )
                for blk in range(SQB):
                    nc.scalar.activation(
                        dsts[p][:, blk * 512:(blk + 1) * 512], pstiles[blk][:],
                        Act.Identity, bias=b_list[p][:, 0:1],
                    )

        # ---- v projection -> 16 tiles [128 tok, 256] bf16 ----
        v_t = []
        for tt in range(SKT):
            pspool, pstag = ((ps_pv, "mm"), (ps_dn, "mm"),
                             (ps_sh, "sh"), (ps_pv, "mm"))[tt % 4]
            ps = pspool.tile([128, DKC], f32, tag=pstag, name="vps")
            for k in range(KT):
                nc.tensor.matmul(
                    ps[:, 0:DKC], lhsT=xv_t[k][:, tt * 128:(tt + 1) * 128],
                    rhs=wv_t[k][:], start=(k == 0), stop=False,
                )
            nc.tensor.matmul(
                ps[:, 0:DKC], lhsT=ones_row[0:1, :], rhs=bv_t[:],
                start=False, stop=True,
            )
            vt = vpool.tile([128, DKC], bf16, tag="v")
            nc.vector.tensor_copy(vt[:], ps[:, 0:DKC])
            v_t.append(vt)

        # ---- attention + Wo ----
        attnT = [atpool.tile([128, S], bf16, tag=f"attnT{p}", name=f"attnT{p}")
                 for p in range(NPAIR)]
        wo_pending = []    # (m, n) groups whose attnT deps are satisfied
        norm_pending = []  # deferred normalization closures

        def emit_wo(ms, pspool, pstag):
            for m in ms:
                ot = opool.tile([128, D], bf16, tag="ot", name="ot")
                for n in range(2):
                    ps = pspool.tile([128, 512], f32, tag=pstag, name="wops")
                    for p in range(NPAIR):
                        nc.tensor.matmul(
                            ps[:], lhsT=attnT[p][:, m * 128:(m + 1) * 128],
                            rhs=wo_t[p][:, n * 512:(n + 1) * 512],
                            start=(p == 0), stop=(p == NPAIR - 1),
                        )
                    nc.vector.tensor_copy(ot[:, n * 512:(n + 1) * 512], ps[:])
                nc.sync.dma_start(out_ext[m * 128:(m + 1) * 128, :], ot[:])

        def do_norm(p, sq, pv_sb, rec_b):
            # rb = broadcast of rec rows (64p, 64p+32) over dk rows; then
            # attnT[:, sq block] = pv_sb * rb
            rbp = ps_sh.tile([128, 512], f32, tag="sh", name="rbp")
            nc.tensor.matmul(rbp[:], lhsT=e97[p][:], rhs=rec_b[0:97, :],
                             start=True, stop=True)
            nc.vector.tensor_mul(
                attnT[p][:, sq * 512:(sq + 1) * 512], pv_sb[:], rbp[:])

        def flush_norm():
            while norm_pending:
                do_norm(*norm_pending.pop(0))

        for sq in range(SQB):
            qs = slice(sq * 512, (sq + 1) * 512)
            dn = ps_dn.tile([128, 512], f32, tag="mm", name="dn")
            pv_sbs = []
            for p in range(NPAIR):
                xps = ps_pv.tile([128, 512], f32, tag="mm", name="xps")

                # scores for step sk, emitted 2 steps ahead of their
                # consumers so the ACT exp stream never starves the PE
                # (and vice versa)
                sc_tiles = {}

                def emit_scores(sk):
                    sc = ps_sc.tile([128, 1024], f32, tag="sc", name="sc")
                    for h in range(2):
                        hp = h * 64
                        nc.tensor.matmul(
                            sc[:, h * 512:(h + 1) * 512],
                            lhsT=kTt[p][hp:hp + 64, sk * 128:(sk + 1) * 128],
                            rhs=qT[p][hp:hp + 64, qs],
                            start=True, stop=True,
                        )
                    sc_tiles[sk] = sc

                emit_scores(0)
                emit_scores(1)
                for sk in range(SKT):
                    sc = sc_tiles.pop(sk)
                    pt = ptpool.tile([128, 1024], bf16, tag="pt")
                    nc.scalar.activation(pt[:], sc[:], Act.Exp, scale=0.125)
                    if sk + 2 < SKT:
                        emit_scores(sk + 2)
                    for h in range(2):
                        nc.tensor.matmul(
                            xps[h * 64:(h + 1) * 64, :],
                            lhsT=v_t[sk][:, p * 128 + h * 64:p * 128 + (h + 1) * 64],
                            rhs=pt[:, h * 512:(h + 1) * 512],
                            start=(sk == 0), stop=(sk == SKT - 1),
                        )
                    for h in range(2):
                        r = p * 64 + h * 32
                        nc.tensor.matmul(
                            dn[r:r + 1, :], lhsT=ones_col[:, 0:1],
                            rhs=pt[:, h * 512:(h + 1) * 512],
                            start=(sk == 0), stop=(sk == SKT - 1),
                            tile_position=(0, r),
                        )
                    # deferred work from the previous sq, placed where its
                    # inputs are long since ready so the PE never stalls
                    if sk == 4:
                        flush_norm()
                    if sk >= 8 and wo_pending:
                        emit_wo([wo_pending.pop(0)], ps_sh, "sh")
                # copy PV out of psum promptly so the next sq can reuse it
                pv_sb = pvsb.tile([128, 512], f32, tag="pvsb", name="pvsb")
                nc.vector.tensor_copy(pv_sb[:], xps[:])
                pv_sbs.append(pv_sb)

            # denominators -> batched fast reciprocal (both pairs at once)
            den_sb = smpool.tile([97, 512], f32, tag="densb", name="densb")
            nc.vector.tensor_copy(den_sb[:], dn[0:97, :])
            rec_f = smpool.tile([97, 512], f32, tag="recf", name="recf")
            nc.vector.reciprocal_approx_fast(rec_f[:], den_sb[:])
            rec_b = smpool.tile([97, 512], bf16, tag="recb", name="recb")
            nc.vector.tensor_copy(rec_b[:], rec_f[:])
            for p in range(NPAIR):
                norm_pending.append((p, sq, pv_sbs[p], rec_b))

            wo_pending.extend(range(sq * 4, (sq + 1) * 4))

        # drain: last sq's normalization + remaining Wo via the (now idle)
        # scores pool for 2-deep pipelining
        flush_norm()
        emit_wo(wo_pending, ps_sc, "sc")

    nc.compile()
    return nc


def _get_nc():
    if "nc" not in _cache:
        _cache["nc"] = _build()
    return _cache["nc"]


def kernel(query, key, value, Wq, bq, Wk, bk, Wv, bv, Wo, bo, trace=False):
    import ml_dtypes
    from concourse.bass_utils import run_bass_kernel_spmd

    nc = _get_nc()
    bf = ml_dtypes.bfloat16

    q = np.asarray(query, np.float32)
    k = np.asarray(key, np.float32)
    v = np.asarray(value, np.float32)
    xT = {}
    for nm, x in (("q", q), ("k", k), ("v", v)):
        for b in range(B):
            xT[(nm, b)] = np.ascontiguousarray(x[b].T).astype(bf)
    Wq = np.asarray(Wq, np.float32).astype(bf)
    Wk = np.asarray(Wk, np.float32).astype(bf)
    Wv = np.asarray(Wv, np.float32).astype(bf)
    Wo = np.asarray(Wo, np.float32).astype(bf)
    bqf = np.asarray(bq, np.float32)
    bkf = np.asarray(bk, np.float32)
    bvf = np.asarray(bv, np.float32).astype(bf)

    in_maps = []
    for r in range(NCORES):
        b, g = divmod(r, GC)
        sl = slice(g * DKC, (g + 1) * DKC)
        in_maps.append({
            "xqT": xT[("q", b)], "xkT": xT[("k", b)], "xvT": xT[("v", b)],
            "wq": np.ascontiguousarray(Wq[:, sl]),
            "wk": np.ascontiguousarray(Wk[:, sl]),
            "wv": np.ascontiguousarray(Wv[:, sl]),
            "wo": np.ascontiguousarray(Wo[sl, :]),
            "bq": np.ascontiguousarray(bqf[sl, None]),
            "bk": np.ascontiguousarray(bkf[sl, None]),
            "bv": np.ascontiguousarray(bvf[None, sl]),
        })

    res = run_bass_kernel_spmd(nc, in_maps, list(range(NCORES)), trace=trace)
    _cache["last_results"] = res

    bo = np.asarray(bo, np.float32)
    out = np.empty((B, S, D), np.float32)
    for b in range(B):
        acc = np.zeros((S, D), np.float32)
        for g in range(GC):
            acc += np.asarray(res.results[b * GC + g]["out"], np.float32)
        out[b] = acc + bo[None, :]
    return out
